# revision 2
# baseline (speedup 1.0000x reference)
"""Embedding lookup (gather of rows) distributed over 8 NeuronCores.

Full problem: x:[1, 8192] int token ids, weights:[50257, 768] f32.
Output: weights[x[0]] -> [8192, 768] f32.

Sharding: data-parallel over the sequence dim. Each of the 8 cores gets
1024 token ids plus a full replica of the embedding table and gathers its
own rows with indirect DMA (DRAM -> SBUF; the HW consumes one index per
partition per indirect DMA, so each gather moves 128 rows), then writes
its contiguous [1024, 768] output slice back to DRAM. No collectives;
the host concatenates the 8 slices.
"""

import numpy as np

import concourse.bass as bass
import concourse.mybir as mybir
import concourse.tile as tile
from concourse import bacc
from concourse.bass_utils import run_bass_kernel_spmd

VOCAB = 50257
EMBED = 768
SEQ = 8192
N_CORES = 8
TOK_PER_CORE = SEQ // N_CORES  # 1024
J = TOK_PER_CORE // 128  # 8 gathers of 128 rows each

# Gathers are grouped so the write-out of one group overlaps the gathers of
# the next.
GROUPS = 4
JPG = J // GROUPS  # gathers per group


def build_nc():
    nc = bacc.Bacc(
        "TRN2", target_bir_lowering=False, debug=False, num_devices=N_CORES
    )
    idx = nc.dram_tensor(
        "idx", [1, TOK_PER_CORE], mybir.dt.int32, kind="ExternalInput"
    )
    w = nc.dram_tensor("w", [VOCAB, EMBED], mybir.dt.float32, kind="ExternalInput")
    out = nc.dram_tensor(
        "out", [TOK_PER_CORE, EMBED], mybir.dt.float32, kind="ExternalOutput"
    )
    # partition p owns tokens p*J .. p*J+J-1 -> out rows p*J+j
    out_pjd = out.ap().rearrange("(p j) d -> p (j d)", p=128)

    with tile.TileContext(nc) as tc:
        with (
            tc.tile_pool(name="idxp", bufs=1) as idxp,
            tc.tile_pool(name="gp", bufs=2) as gp,
        ):
            idx_t = idxp.tile([128, J], mybir.dt.int32)
            nc.sync.dma_start(
                idx_t[:], idx.ap().rearrange("a (p j) -> (a p) j", p=128)
            )
            for grp in range(GROUPS):
                g = gp.tile([128, JPG * EMBED], mybir.dt.float32)
                for jj in range(JPG):
                    j = grp * JPG + jj
                    nc.gpsimd.indirect_dma_start(
                        out=g[:, jj * EMBED : (jj + 1) * EMBED],
                        out_offset=None,
                        in_=w.ap(),
                        in_offset=bass.IndirectOffsetOnAxis(
                            ap=idx_t[:, j : j + 1], axis=0
                        ),
                    )
                nc.sync.dma_start(
                    out_pjd[:, grp * JPG * EMBED : (grp + 1) * JPG * EMBED],
                    g[:],
                )

    nc.compile()
    return nc


def kernel(x, weights):
    x_np = np.ascontiguousarray(np.asarray(x).reshape(-1).astype(np.int32))
    w_np = np.ascontiguousarray(np.asarray(weights), dtype=np.float32)
    assert x_np.shape == (SEQ,) and w_np.shape == (VOCAB, EMBED)

    nc = build_nc()
    in_maps = [
        {
            "idx": x_np[k * TOK_PER_CORE : (k + 1) * TOK_PER_CORE].reshape(
                1, TOK_PER_CORE
            ),
            "w": w_np,
        }
        for k in range(N_CORES)
    ]
    res = run_bass_kernel_spmd(nc, in_maps, core_ids=list(range(N_CORES)))
    return np.concatenate([r["out"] for r in res.results], axis=0)


# revision 5
# speedup vs baseline: 1.1039x; 1.1039x over previous
"""Embedding lookup (gather of rows) distributed over 8 NeuronCores.

Full problem: x:[1, 8192] int token ids, weights:[50257, 768] f32.
Output: weights[x[0]] -> [8192, 768] f32.

Sharding: data-parallel over the sequence dim. Each of the 8 cores gets
1024 token ids plus a full replica of the embedding table and gathers its
own rows with indirect DMA (DRAM -> SBUF; the HW consumes one index per
partition per indirect DMA, so each gather moves 128 rows), then writes
its contiguous [1024, 768] output slice back to DRAM. No collectives;
the host concatenates the 8 slices.

Raw Bass (no TileContext): the kernel is two engine streams (gpsimd for
the SWDGE gathers, sync/SP for the HWDGE idx-load + write-outs) with
manual semaphores, avoiding Tile's all-engine start/stop barriers.
"""

import numpy as np

import concourse.bass as bass
import concourse.mybir as mybir
from concourse import bacc
from concourse.bass_utils import run_bass_kernel_spmd

VOCAB = 50257
EMBED = 768
SEQ = 8192
N_CORES = 8
TOK_PER_CORE = SEQ // N_CORES  # 1024
J = TOK_PER_CORE // 128  # 8 gathers of 128 rows each

# Write-outs are grouped so the write of one group overlaps later gathers.
GROUPS = 4
JPG = J // GROUPS  # gathers per write group


def build_nc():
    nc = bacc.Bacc(
        "TRN2", target_bir_lowering=False, debug=False, num_devices=N_CORES
    )
    idx = nc.dram_tensor(
        "idx", [1, TOK_PER_CORE], mybir.dt.int32, kind="ExternalInput"
    )
    w = nc.dram_tensor("w", [VOCAB, EMBED], mybir.dt.float32, kind="ExternalInput")
    out = nc.dram_tensor(
        "out", [TOK_PER_CORE, EMBED], mybir.dt.float32, kind="ExternalOutput"
    )
    # partition p owns tokens p*J .. p*J+J-1 -> out rows p*J+j
    out_pjd = out.ap().rearrange("(p j) d -> p (j d)", p=128)
    idx_re = idx.ap().rearrange("a (p j) -> (a p) j", p=128)

    with (
        nc.sbuf_tensor("idx_sb", [128, J], mybir.dt.int32) as idx_sb,
        nc.sbuf_tensor("gbuf", [128, J * EMBED], mybir.dt.float32) as gbuf,
        nc.semaphore("dsem") as dsem,
    ):
        gsems = [nc.alloc_semaphore(f"gsem{g}") for g in range(GROUPS)]

        with nc.Block(no_gpsimd_drain=True) as block:

            @block.gpsimd
            def _(gpsimd):
                gpsimd.wait_ge(dsem, 16)  # idx ids in SBUF
                for j in range(J):
                    gpsimd.indirect_dma_start(
                        out=gbuf[:, j * EMBED : (j + 1) * EMBED],
                        out_offset=None,
                        in_=w.ap(),
                        in_offset=bass.IndirectOffsetOnAxis(
                            ap=idx_sb[:, j : j + 1], axis=0
                        ),
                    ).then_inc(gsems[j // JPG], 16)

            @block.sync
            def _(sync):
                sync.dma_start(idx_sb[:], idx_re).then_inc(dsem, 16)
                for grp in range(GROUPS):
                    sync.wait_ge(gsems[grp], JPG * 16)
                    sync.dma_start(
                        out_pjd[:, grp * JPG * EMBED : (grp + 1) * JPG * EMBED],
                        gbuf[:, grp * JPG * EMBED : (grp + 1) * JPG * EMBED],
                    ).then_inc(dsem, 16)
                sync.wait_ge(dsem, 16 + GROUPS * 16)

        # After the all-engine barrier: reset semaphores for NEFF re-execution.
        nc.sync.sem_clear(dsem)
        for g in range(GROUPS):
            nc.sync.sem_clear(gsems[g])

    nc.compile()
    return nc


def kernel(x, weights):
    x_np = np.ascontiguousarray(np.asarray(x).reshape(-1).astype(np.int32))
    w_np = np.ascontiguousarray(np.asarray(weights), dtype=np.float32)
    assert x_np.shape == (SEQ,) and w_np.shape == (VOCAB, EMBED)

    nc = build_nc()
    in_maps = [
        {
            "idx": x_np[k * TOK_PER_CORE : (k + 1) * TOK_PER_CORE].reshape(
                1, TOK_PER_CORE
            ),
            "w": w_np,
        }
        for k in range(N_CORES)
    ]
    res = run_bass_kernel_spmd(nc, in_maps, core_ids=list(range(N_CORES)))
    return np.concatenate([r["out"] for r in res.results], axis=0)


# revision 6
# speedup vs baseline: 1.1173x; 1.0121x over previous
"""Embedding lookup (gather of rows) distributed over 8 NeuronCores.

Full problem: x:[1, 8192] int token ids, weights:[50257, 768] f32.
Output: weights[x[0]] -> [8192, 768] f32.

Sharding: data-parallel over the sequence dim. Each of the 8 cores gets
1024 token ids plus a full replica of the embedding table and gathers its
own rows with indirect DMA (DRAM -> SBUF; the HW consumes one index per
partition per indirect DMA, so each gather moves 128 rows), then writes
its contiguous [1024, 768] output slice back to DRAM. No collectives;
the host concatenates the 8 slices.

Raw Bass (no TileContext): the kernel is two engine streams (gpsimd for
the SWDGE gathers, sync/SP for the HWDGE idx-load + write-outs) with
manual semaphores, avoiding Tile's all-engine start/stop barriers.
"""

import numpy as np

import concourse.bass as bass
import concourse.mybir as mybir
from concourse import bacc
from concourse.bass_utils import run_bass_kernel_spmd

VOCAB = 50257
EMBED = 768
SEQ = 8192
N_CORES = 8
TOK_PER_CORE = SEQ // N_CORES  # 1024
J = TOK_PER_CORE // 128  # 8 gathers of 128 rows each

# Write-outs are grouped so the write of one group overlaps later gathers.
GROUPS = 8
JPG = J // GROUPS  # gathers per write group


def build_nc():
    nc = bacc.Bacc(
        "TRN2", target_bir_lowering=False, debug=False, num_devices=N_CORES
    )
    idx = nc.dram_tensor(
        "idx", [1, TOK_PER_CORE], mybir.dt.int32, kind="ExternalInput"
    )
    w = nc.dram_tensor("w", [VOCAB, EMBED], mybir.dt.float32, kind="ExternalInput")
    out = nc.dram_tensor(
        "out", [TOK_PER_CORE, EMBED], mybir.dt.float32, kind="ExternalOutput"
    )
    # partition p owns tokens p*J .. p*J+J-1 -> out rows p*J+j
    out_pjd = out.ap().rearrange("(p j) d -> p (j d)", p=128)
    idx_re = idx.ap().rearrange("a (p j) -> (a p) j", p=128)

    with (
        nc.sbuf_tensor("idx_sb", [128, J], mybir.dt.int32) as idx_sb,
        nc.sbuf_tensor("gbuf", [128, J * EMBED], mybir.dt.float32) as gbuf,
        nc.semaphore("dsem") as dsem,
    ):
        gsems = [nc.alloc_semaphore(f"gsem{g}") for g in range(GROUPS)]

        with nc.Block(no_gpsimd_drain=True) as block:

            @block.gpsimd
            def _(gpsimd):
                gpsimd.wait_ge(dsem, 16)  # idx ids in SBUF
                for j in range(J):
                    gpsimd.indirect_dma_start(
                        out=gbuf[:, j * EMBED : (j + 1) * EMBED],
                        out_offset=None,
                        in_=w.ap(),
                        in_offset=bass.IndirectOffsetOnAxis(
                            ap=idx_sb[:, j : j + 1], axis=0
                        ),
                    ).then_inc(gsems[j // JPG], 16)

            @block.sync
            def _(sync):
                sync.dma_start(idx_sb[:], idx_re).then_inc(dsem, 16)
                for grp in range(GROUPS):
                    sync.wait_ge(gsems[grp], JPG * 16)
                    sync.dma_start(
                        out_pjd[:, grp * JPG * EMBED : (grp + 1) * JPG * EMBED],
                        gbuf[:, grp * JPG * EMBED : (grp + 1) * JPG * EMBED],
                    ).then_inc(dsem, 16)
                sync.wait_ge(dsem, 16 + GROUPS * 16)

        # After the all-engine barrier: reset semaphores for NEFF re-execution.
        nc.sync.sem_clear(dsem)
        for g in range(GROUPS):
            nc.sync.sem_clear(gsems[g])

    nc.compile()
    return nc


def kernel(x, weights):
    x_np = np.ascontiguousarray(np.asarray(x).reshape(-1).astype(np.int32))
    w_np = np.ascontiguousarray(np.asarray(weights), dtype=np.float32)
    assert x_np.shape == (SEQ,) and w_np.shape == (VOCAB, EMBED)

    nc = build_nc()
    in_maps = [
        {
            "idx": x_np[k * TOK_PER_CORE : (k + 1) * TOK_PER_CORE].reshape(
                1, TOK_PER_CORE
            ),
            "w": w_np,
        }
        for k in range(N_CORES)
    ]
    res = run_bass_kernel_spmd(nc, in_maps, core_ids=list(range(N_CORES)))
    return np.concatenate([r["out"] for r in res.results], axis=0)


# revision 7
# speedup vs baseline: 1.1970x; 1.0714x over previous
"""Embedding lookup (gather of rows) distributed over 8 NeuronCores.

Full problem: x:[1, 8192] int token ids, weights:[50257, 768] f32.
Output: weights[x[0]] -> [8192, 768] f32.

Sharding: data-parallel over the sequence dim. Each of the 8 cores gets
1024 token ids plus a full replica of the embedding table and gathers its
own rows with indirect DMA (DRAM -> SBUF; the HW consumes one index per
partition per indirect DMA, so each gather moves 128 rows), then writes
its contiguous [1024, 768] output slice back to DRAM. No collectives;
the host concatenates the 8 slices.

Raw Bass, no Block(): the two active engine streams (gpsimd: SWDGE idx
load + gathers; SP: HWDGE write-outs) are emitted straight into the main
block with manual semaphores and no end-of-kernel all-engine barrier, so
the idle engines retire immediately and the runtime's per-engine
semaphore-cleanup epilogue overlaps the data phase instead of
serializing after it. Semaphore reset for re-execution is provided by
that same runtime cleanup (it zeroes all 256 semaphores).
"""

import numpy as np

import concourse.bass as bass
import concourse.mybir as mybir
from concourse import bacc
from concourse.bass_utils import run_bass_kernel_spmd

VOCAB = 50257
EMBED = 768
SEQ = 8192
N_CORES = 8
TOK_PER_CORE = SEQ // N_CORES  # 1024
J = TOK_PER_CORE // 128  # 8 gathers of 128 rows each


def build_nc():
    nc = bacc.Bacc(
        "TRN2", target_bir_lowering=False, debug=False, num_devices=N_CORES
    )
    idx = nc.dram_tensor(
        "idx", [1, TOK_PER_CORE], mybir.dt.int32, kind="ExternalInput"
    )
    w = nc.dram_tensor("w", [VOCAB, EMBED], mybir.dt.float32, kind="ExternalInput")
    out = nc.dram_tensor(
        "out", [TOK_PER_CORE, EMBED], mybir.dt.float32, kind="ExternalOutput"
    )
    # partition p owns tokens p*J .. p*J+J-1 -> out rows p*J+j
    out_pjd = out.ap().rearrange("(p j) d -> p (j d)", p=128)
    idx_re = idx.ap().rearrange("a (p j) -> (a p) j", p=128)

    with (
        nc.sbuf_tensor("idx_sb", [128, J], mybir.dt.int32) as idx_sb,
        nc.sbuf_tensor("gbuf", [128, J * EMBED], mybir.dt.float32) as gbuf,
        nc.semaphore("isem") as isem,
        nc.semaphore("wsem") as wsem,
    ):
        gsems = [nc.alloc_semaphore(f"gsem{g}") for g in range(J)]

        gp = nc.gpsimd
        sp = nc.sync

        # gpsimd stream: idx load (SWDGE) then the 8 gathers.
        gp.dma_start(idx_sb[:], idx_re).then_inc(isem, 16)
        gp.wait_ge(isem, 16)
        for j in range(J):
            gp.indirect_dma_start(
                out=gbuf[:, j * EMBED : (j + 1) * EMBED],
                out_offset=None,
                in_=w.ap(),
                in_offset=bass.IndirectOffsetOnAxis(ap=idx_sb[:, j : j + 1], axis=0),
            ).then_inc(gsems[j], 16)

        # SP stream: per-gather write-outs, then wait for everything.
        for j in range(J):
            sp.wait_ge(gsems[j], 16)
            sp.dma_start(
                out_pjd[:, j * EMBED : (j + 1) * EMBED],
                gbuf[:, j * EMBED : (j + 1) * EMBED],
            ).then_inc(wsem, 16)
        sp.wait_ge(wsem, J * 16)

    nc.compile()
    return nc


def kernel(x, weights):
    x_np = np.ascontiguousarray(np.asarray(x).reshape(-1).astype(np.int32))
    w_np = np.ascontiguousarray(np.asarray(weights), dtype=np.float32)
    assert x_np.shape == (SEQ,) and w_np.shape == (VOCAB, EMBED)

    nc = build_nc()
    in_maps = [
        {
            "idx": x_np[k * TOK_PER_CORE : (k + 1) * TOK_PER_CORE].reshape(
                1, TOK_PER_CORE
            ),
            "w": w_np,
        }
        for k in range(N_CORES)
    ]
    res = run_bass_kernel_spmd(nc, in_maps, core_ids=list(range(N_CORES)))
    return np.concatenate([r["out"] for r in res.results], axis=0)


# revision 10
# speedup vs baseline: 1.2967x; 1.0833x over previous
"""Embedding lookup (gather of rows) distributed over 8 NeuronCores.

Full problem: x:[1, 8192] int token ids, weights:[50257, 768] f32.
Output: weights[x[0]] -> [8192, 768] f32.

Sharding: data-parallel over the sequence dim. Each of the 8 cores gets
1024 token ids plus a full replica of the embedding table and gathers its
own rows with indirect DMA (DRAM -> SBUF; the HW consumes one index per
partition per indirect DMA, so each gather moves 128 rows), then writes
its contiguous [1024, 768] output slice back to DRAM. No collectives;
the host concatenates the 8 slices.

Raw Bass, no Block(): the two active engine streams (gpsimd: SWDGE idx
load + gathers; SP: HWDGE write-outs) are emitted straight into the main
block with manual semaphores and no end-of-kernel all-engine barrier, so
the idle engines retire immediately and the runtime's per-engine
semaphore-cleanup epilogue overlaps the data phase instead of
serializing after it. Semaphore reset for re-execution is provided by
that same runtime cleanup (it zeroes all 256 semaphores).
"""

import numpy as np

import concourse.bass as bass
import concourse.mybir as mybir
from concourse import bacc
from concourse.bass_utils import run_bass_kernel_spmd

VOCAB = 50257
EMBED = 768
SEQ = 8192
N_CORES = 8
TOK_PER_CORE = SEQ // N_CORES  # 1024
J = TOK_PER_CORE // 128  # 8 gathers of 128 rows each


def build_nc():
    nc = bacc.Bacc(
        "TRN2", target_bir_lowering=False, debug=False, num_devices=N_CORES
    )
    idx = nc.dram_tensor(
        "idx", [1, TOK_PER_CORE], mybir.dt.int32, kind="ExternalInput"
    )
    w = nc.dram_tensor("w", [VOCAB, EMBED], mybir.dt.float32, kind="ExternalInput")
    out = nc.dram_tensor(
        "out", [TOK_PER_CORE, EMBED], mybir.dt.float32, kind="ExternalOutput"
    )
    # partition p owns tokens p*J .. p*J+J-1 -> out rows p*J+j
    out_pjd = out.ap().rearrange("(p j) d -> p (j d)", p=128)
    idx_re = idx.ap().rearrange("a (p j) -> (a p) j", p=128)

    with (
        nc.sbuf_tensor("idx_sb", [128, J], mybir.dt.int32) as idx_sb,
        nc.sbuf_tensor("gbuf", [128, J * EMBED], mybir.dt.float32) as gbuf,
        nc.semaphore("isem") as isem,
        nc.semaphore("wsem") as wsem,
    ):
        gsems = [nc.alloc_semaphore(f"gsem{g}") for g in range(J)]

        gp = nc.gpsimd
        sp = nc.sync

        # idx load on SP (HWDGE: lower first-byte latency); gathers on gpsimd.
        sp.dma_start(idx_sb[:], idx_re).then_inc(isem, 16)
        gp.wait_ge(isem, 16)
        for j in range(J):
            gp.indirect_dma_start(
                out=gbuf[:, j * EMBED : (j + 1) * EMBED],
                out_offset=None,
                in_=w.ap(),
                in_offset=bass.IndirectOffsetOnAxis(ap=idx_sb[:, j : j + 1], axis=0),
            ).then_inc(gsems[j], 16)

        # SP stream: per-gather write-outs, then wait for everything.
        for j in range(J):
            sp.wait_ge(gsems[j], 16)
            sp.dma_start(
                out_pjd[:, j * EMBED : (j + 1) * EMBED],
                gbuf[:, j * EMBED : (j + 1) * EMBED],
            ).then_inc(wsem, 16)
        sp.wait_ge(wsem, J * 16)

    # Drop the const-AP prime memsets Bass emits unconditionally in its
    # prologue — nothing in this kernel reads them, and they extend the
    # measured kernel span at the front.
    entry = nc.m.functions[0].blocks[0]
    dead = [
        i
        for i in entry.instructions
        if isinstance(i, mybir.InstMemset)
        and i.outs
        and str(getattr(i.outs[0], "memref", "")).startswith("const-")
    ]
    for i in dead:
        entry.instructions.remove(i)

    nc.compile()
    return nc


def kernel(x, weights):
    x_np = np.ascontiguousarray(np.asarray(x).reshape(-1).astype(np.int32))
    w_np = np.ascontiguousarray(np.asarray(weights), dtype=np.float32)
    assert x_np.shape == (SEQ,) and w_np.shape == (VOCAB, EMBED)

    nc = build_nc()
    in_maps = [
        {
            "idx": x_np[k * TOK_PER_CORE : (k + 1) * TOK_PER_CORE].reshape(
                1, TOK_PER_CORE
            ),
            "w": w_np,
        }
        for k in range(N_CORES)
    ]
    res = run_bass_kernel_spmd(nc, in_maps, core_ids=list(range(N_CORES)))
    return np.concatenate([r["out"] for r in res.results], axis=0)


# revision 11
# speedup vs baseline: 1.3358x; 1.0302x over previous
"""Embedding lookup (gather of rows) distributed over 8 NeuronCores.

Full problem: x:[1, 8192] int token ids, weights:[50257, 768] f32.
Output: weights[x[0]] -> [8192, 768] f32.

Sharding: data-parallel over the sequence dim. Each of the 8 cores gets
1024 token ids plus a full replica of the embedding table and gathers its
own rows with indirect DMA (DRAM -> SBUF; the HW consumes one index per
partition per indirect DMA, so each gather moves 128 rows), then writes
its contiguous [1024, 768] output slice back to DRAM. No collectives;
the host concatenates the 8 slices.

Raw Bass, no Block(): the two active engine streams (gpsimd: SWDGE idx
load + gathers; SP: HWDGE write-outs) are emitted straight into the main
block with manual semaphores and no end-of-kernel all-engine barrier, so
the idle engines retire immediately and the runtime's per-engine
semaphore-cleanup epilogue overlaps the data phase instead of
serializing after it. Semaphore reset for re-execution is provided by
that same runtime cleanup (it zeroes all 256 semaphores).
"""

import numpy as np

import concourse.bass as bass
import concourse.mybir as mybir
from concourse import bacc
from concourse.bass_utils import run_bass_kernel_spmd

VOCAB = 50257
EMBED = 768
SEQ = 8192
N_CORES = 8
TOK_PER_CORE = SEQ // N_CORES  # 1024
J = TOK_PER_CORE // 128  # 8 gathers of 128 rows each


def build_nc():
    nc = bacc.Bacc(
        "TRN2", target_bir_lowering=False, debug=False, num_devices=N_CORES
    )
    idx = nc.dram_tensor(
        "idx", [1, TOK_PER_CORE], mybir.dt.int32, kind="ExternalInput"
    )
    w = nc.dram_tensor("w", [VOCAB, EMBED], mybir.dt.float32, kind="ExternalInput")
    out = nc.dram_tensor(
        "out", [TOK_PER_CORE, EMBED], mybir.dt.float32, kind="ExternalOutput"
    )
    # partition p owns tokens p*J .. p*J+J-1 -> out rows p*J+j
    out_pjd = out.ap().rearrange("(p j) d -> p (j d)", p=128)
    idx_re = idx.ap().rearrange("a (p j) -> (a p) j", p=128)

    with (
        nc.sbuf_tensor("idx_sb", [128, J], mybir.dt.int32) as idx_sb,
        nc.sbuf_tensor("gbuf", [128, J * EMBED], mybir.dt.float32) as gbuf,
        nc.semaphore("isem") as isem,
        nc.semaphore("wsem") as wsem,
    ):
        gsems = [nc.alloc_semaphore(f"gsem{g}") for g in range(J)]

        gp = nc.gpsimd
        sp = nc.sync

        # idx load on SP (HWDGE: lower first-byte latency); gathers on gpsimd.
        sp.dma_start(idx_sb[:], idx_re).then_inc(isem, 16)
        gp.wait_ge(isem, 16)
        for j in range(J):
            gp.indirect_dma_start(
                out=gbuf[:, j * EMBED : (j + 1) * EMBED],
                out_offset=None,
                in_=w.ap(),
                in_offset=bass.IndirectOffsetOnAxis(ap=idx_sb[:, j : j + 1], axis=0),
            ).then_inc(gsems[j], 16)

        # Write-outs alternate between the two HWDGE queues (SP and ACT) so
        # descriptor rings and completion receipts don't pile on one queue.
        act = nc.scalar
        for j in range(J):
            eng = sp if j % 2 == 0 else act
            eng.wait_ge(gsems[j], 16)
            eng.dma_start(
                out_pjd[:, j * EMBED : (j + 1) * EMBED],
                gbuf[:, j * EMBED : (j + 1) * EMBED],
            ).then_inc(wsem, 16)
        sp.wait_ge(wsem, J * 16)

    # Drop the const-AP prime memsets Bass emits unconditionally in its
    # prologue — nothing in this kernel reads them, and they extend the
    # measured kernel span at the front.
    entry = nc.m.functions[0].blocks[0]
    dead = [
        i
        for i in entry.instructions
        if isinstance(i, mybir.InstMemset)
        and i.outs
        and str(getattr(i.outs[0], "memref", "")).startswith("const-")
    ]
    for i in dead:
        entry.instructions.remove(i)

    nc.compile()
    return nc


def kernel(x, weights):
    x_np = np.ascontiguousarray(np.asarray(x).reshape(-1).astype(np.int32))
    w_np = np.ascontiguousarray(np.asarray(weights), dtype=np.float32)
    assert x_np.shape == (SEQ,) and w_np.shape == (VOCAB, EMBED)

    nc = build_nc()
    in_maps = [
        {
            "idx": x_np[k * TOK_PER_CORE : (k + 1) * TOK_PER_CORE].reshape(
                1, TOK_PER_CORE
            ),
            "w": w_np,
        }
        for k in range(N_CORES)
    ]
    res = run_bass_kernel_spmd(nc, in_maps, core_ids=list(range(N_CORES)))
    return np.concatenate([r["out"] for r in res.results], axis=0)


# revision 12
# speedup vs baseline: 1.3631x; 1.0204x over previous
"""Embedding lookup (gather of rows) distributed over 8 NeuronCores.

Full problem: x:[1, 8192] int token ids, weights:[50257, 768] f32.
Output: weights[x[0]] -> [8192, 768] f32.

Sharding: data-parallel over the sequence dim. Each of the 8 cores gets
1024 token ids plus a full replica of the embedding table and gathers its
own rows with indirect DMA (DRAM -> SBUF; the HW consumes one index per
partition per indirect DMA, so each gather moves 128 rows), then writes
its contiguous [1024, 768] output slice back to DRAM. No collectives;
the host concatenates the 8 slices.

Raw Bass, no Block(): the two active engine streams (gpsimd: SWDGE idx
load + gathers; SP: HWDGE write-outs) are emitted straight into the main
block with manual semaphores and no end-of-kernel all-engine barrier, so
the idle engines retire immediately and the runtime's per-engine
semaphore-cleanup epilogue overlaps the data phase instead of
serializing after it. Semaphore reset for re-execution is provided by
that same runtime cleanup (it zeroes all 256 semaphores).
"""

import numpy as np

import concourse.bass as bass
import concourse.mybir as mybir
from concourse import bacc
from concourse.bass_utils import run_bass_kernel_spmd

VOCAB = 50257
EMBED = 768
SEQ = 8192
N_CORES = 8
TOK_PER_CORE = SEQ // N_CORES  # 1024
J = TOK_PER_CORE // 128  # 8 gathers of 128 rows each


def build_nc():
    nc = bacc.Bacc(
        "TRN2", target_bir_lowering=False, debug=False, num_devices=N_CORES
    )
    idx = nc.dram_tensor(
        "idx", [1, TOK_PER_CORE], mybir.dt.int32, kind="ExternalInput"
    )
    w = nc.dram_tensor("w", [VOCAB, EMBED], mybir.dt.float32, kind="ExternalInput")
    out = nc.dram_tensor(
        "out", [TOK_PER_CORE, EMBED], mybir.dt.float32, kind="ExternalOutput"
    )
    # partition p owns tokens p*J .. p*J+J-1 -> out rows p*J+j
    out_pjd = out.ap().rearrange("(p j) d -> p (j d)", p=128)
    idx_re = idx.ap().rearrange("a (p j) -> (a p) j", p=128)

    with (
        nc.sbuf_tensor("idx_sb", [128, J], mybir.dt.int32) as idx_sb,
        nc.sbuf_tensor("gbuf", [128, J * EMBED], mybir.dt.float32) as gbuf,
        nc.semaphore("isem") as isem,
        nc.semaphore("wsem") as wsem,
    ):
        gsems = [nc.alloc_semaphore(f"gsem{g}") for g in range(J)]

        gp = nc.gpsimd
        sp = nc.sync

        # idx load on SP (HWDGE: lower first-byte latency); gathers on gpsimd.
        sp.dma_start(idx_sb[:], idx_re).then_inc(isem, 16)
        gp.wait_ge(isem, 16)
        for j in range(J):
            gp.indirect_dma_start(
                out=gbuf[:, j * EMBED : (j + 1) * EMBED],
                out_offset=None,
                in_=w.ap(),
                in_offset=bass.IndirectOffsetOnAxis(ap=idx_sb[:, j : j + 1], axis=0),
            ).then_inc(gsems[j], 16)

        # Write-outs pair two gathers each (6KB contiguous runs in DRAM) and
        # alternate between the two HWDGE queues (SP and ACT) so descriptor
        # rings and completion receipts don't pile on one queue.
        act = nc.scalar
        JPG = 2
        for g in range(J // JPG):
            eng = sp if g % 2 == 0 else act
            for jj in range(JPG):
                eng.wait_ge(gsems[g * JPG + jj], 16)
            eng.dma_start(
                out_pjd[:, g * JPG * EMBED : (g + 1) * JPG * EMBED],
                gbuf[:, g * JPG * EMBED : (g + 1) * JPG * EMBED],
            ).then_inc(wsem, 16)
        sp.wait_ge(wsem, (J // JPG) * 16)

    # Drop the const-AP prime memsets Bass emits unconditionally in its
    # prologue — nothing in this kernel reads them, and they extend the
    # measured kernel span at the front.
    entry = nc.m.functions[0].blocks[0]
    dead = [
        i
        for i in entry.instructions
        if isinstance(i, mybir.InstMemset)
        and i.outs
        and str(getattr(i.outs[0], "memref", "")).startswith("const-")
    ]
    for i in dead:
        entry.instructions.remove(i)

    nc.compile()
    return nc


def kernel(x, weights):
    x_np = np.ascontiguousarray(np.asarray(x).reshape(-1).astype(np.int32))
    w_np = np.ascontiguousarray(np.asarray(weights), dtype=np.float32)
    assert x_np.shape == (SEQ,) and w_np.shape == (VOCAB, EMBED)

    nc = build_nc()
    in_maps = [
        {
            "idx": x_np[k * TOK_PER_CORE : (k + 1) * TOK_PER_CORE].reshape(
                1, TOK_PER_CORE
            ),
            "w": w_np,
        }
        for k in range(N_CORES)
    ]
    res = run_bass_kernel_spmd(nc, in_maps, core_ids=list(range(N_CORES)))
    return np.concatenate([r["out"] for r in res.results], axis=0)
